# revision 10
# baseline (speedup 1.0000x reference)
"""Causal self-attention (B=4, T=2048, C=1024, H=16) on 8 trn2 cores.

Sharding: core c = (batch b = c//2, head-group g = c%2). Each core computes
attention for 8 heads of one batch plus the partial output projection for its
512-channel slice; the host sums the two partials per batch and adds b_proj.

All operands are bf16 (converted host-side): halves input DMA, removes cast
traffic, keeps every matmul at 1 cycle/row. PSUM accumulation stays fp32.

The kernel is a single software-pipelined pass. Phase 1a builds x^T via PE
transposes (with the f=0 QKV feature block interleaved into DMA gaps). After
that, QKV feature-tile groups, V token-tiles, attention pairs, projections
and softmax divides are emitted in one dependency-ordered stream (slabs
ascending) so the scalar engine's exp — the kernel's busiest single load —
starts ~10us in and never starves, while the PE fills every exp-bound gap
with QKV / V / projection matmuls. Scores, QKV groups and V tiles share one
2-slot PSUM pool; PV accumulators use the other (8 banks total).

Per (head-pair, query-slab): S^T = K^T.T @ Q^T (bf16 K=64 pair, concurrent
via row-group tile_position, causally narrowed), P^T = exp(S^T/8) on ACT
(bf16 out), 0/1 mask on the diagonal window, Y^T_aug = Vaug.T @ P^T
accumulated over key tiles with row 64 = softmax denominators. The divide
(reciprocal_approx_fast + gpsimd partition_broadcast + quadrant muls) is
deferred into the next pair; the previous slab's projection is emitted
before it and DMAs straight from PSUM.
"""

import os
import sys

import numpy as np

B, T, C, H = 4, 2048, 1024, 16
HD = C // H          # 64
G = 2                # head groups (cores per batch)
CL = C // G          # 512 local channels
HL = H // G          # 8 local heads
P = 128
NCC = C // P         # 8 contraction chunks over C
NTT = T // P         # 16 token tiles
SLAB = 512
NS = T // SLAB       # 4 query slabs
NPAIR = HL // 2      # 4 head pairs
E = HD + 1           # 65: head dim + ones column

_CACHE: dict = {}


def _ensure_paths():
    try:
        import concourse  # noqa: F401
    except ImportError:
        for p in ("/opt/trn_rl_repo", "/root/.axon_site/_ro/trn_rl_repo"):
            if os.path.isdir(p) and p not in sys.path:
                sys.path.insert(0, p)
        import concourse  # noqa: F401


def _phase1a(nc, tc, consts, qkT, xT, xTb, wvr_, dtypes, misc):
    """x^T via PE transpose; f=0 QKV interleaved; V/proj weights prefetch."""
    f32, bf16 = dtypes
    ts, Alu = misc["ts"], misc["Alu"]
    ident, bqk = consts["ident"], consts["bqk"]

    w0p = tc.alloc_tile_pool(name="w0p", bufs=2)
    ap0 = tc.alloc_tile_pool(name="ap0", bufs=2, space="PSUM")
    wr0 = w0p.tile([P, C], bf16, tag="wt0", name="wt0")
    with (
        tc.tile_pool(name="xst", bufs=3) as xst,
        tc.tile_pool(name="tps", bufs=4, space="PSUM") as tps,
    ):
        for tt2 in range(NTT // 2):
            xt_ = xst.tile([P, 2 * C], bf16)
            if tt2 == 0:
                # first pair: split across both queues so the two tiles
                # transfer in parallel and the PE starts sooner
                nc.sync.dma_start(xt_[:, 0:C], misc["x_d"][ts(0, P), :])
                nc.gpsimd.dma_start(xt_[:, C : 2 * C], misc["x_d"][ts(1, P), :])
            else:
                nc.sync.dma_start(
                    xt_[:].rearrange("p (two c) -> p two c", two=2),
                    misc["x_d"][ts(tt2, 2 * P), :].rearrange(
                        "(two p) c -> p two c", p=P
                    ),
                )
            if tt2 == 0:
                nc.gpsimd.dma_start(
                    wr0[:].rearrange("p (cc f) -> p cc f", cc=NCC),
                    misc["wqkv_d"][:, 0:P].rearrange("(cc p) f -> p cc f", p=P),
                )
                wpt = consts["wproj"]
                nc.gpsimd.dma_start(
                    wpt[:].rearrange("p (cc f) -> p cc f", cc=4),
                    misc["wproj_d"][:, :].rearrange("(cc p) f -> p cc f", p=P),
                )
                # V weights on the gpsimd queue: ready before the first
                # V token-tile right after 1a
                for h_ in range(4):
                    nc.gpsimd.dma_start(
                        wvr_[:, ts(h_, 2 * CL)].rearrange("p (cc f) -> p cc f", cc=2),
                        misc["wqkv_d"][ts(h_, 2 * P), 2 * CL : 3 * CL].rearrange(
                            "(cc p) f -> p cc f", p=P
                        ),
                    )
            for sub in range(2):
                tt = 2 * tt2 + sub
                for q4 in range(2):
                    pt_ = tps.tile([P, 4 * P], bf16)
                    for c4 in range(4):
                        cc = 4 * q4 + c4
                        nc.tensor.transpose(
                            pt_[:, ts(c4, P)],
                            xt_[:, sub * C + cc * P : sub * C + (cc + 1) * P],
                            ident[:],
                        )
                    nc.vector.tensor_copy(
                        xTb[:].rearrange("p (cc t) -> p cc t", cc=NCC)[
                            :, 4 * q4 : 4 * q4 + 4, ts(tt, P)
                        ],
                        pt_[:].rearrange("p (c4 t) -> p c4 t", c4=4),
                    )
            if tt2 % 2 == 1:
                s0 = tt2 // 2
                ps0 = ap0.tile([P, SLAB], f32, tag="ps0", name="ps0")
                for cc in range(NCC):
                    nc.tensor.matmul(
                        ps0[:],
                        wr0[:, ts(cc, P)],
                        xT[cc][:, ts(s0, SLAB)],
                        start=(cc == 0),
                        stop=(cc == NCC - 1),
                    )
                nc.vector.tensor_tensor(
                    out=qkT[0][:, ts(s0, SLAB)],
                    in0=ps0[:],
                    in1=bqk[0][:].to_broadcast((P, SLAB)),
                    op=Alu.add,
                )
    ap0.release()
    w0p.release()


def _pipeline(nc, tc, consts, qkT, vaug, xT, wvr_, dtypes, misc):
    f32, bf16 = dtypes
    ts, Alu, AF = misc["ts"], misc["Alu"], misc["AF"]
    mask, bqk, bvf, onesf = (consts["mask"], consts["bqk"], consts["bvf"],
                             consts["onesf"])
    wpr_ = consts["wproj"][:]

    with (
        tc.tile_pool(name="wst", bufs=1) as wst,
        tc.tile_pool(name="ptp", bufs=6) as ptp,
        tc.tile_pool(name="ytp", bufs=10) as ytp,
        tc.tile_pool(name="dvp", bufs=2) as dvp,
        tc.tile_pool(name="rbp", bufs=2) as rbp,
        tc.tile_pool(name="osb", bufs=3) as osb,
        tc.tile_pool(name="spp", bufs=2, space="PSUM") as spp,
        tc.tile_pool(name="ypp", bufs=2, space="PSUM") as ypp,
    ):
        # the 7 remaining QKV feature tiles, fetched once, kept resident
        ftile = {}
        for f in (4, 1, 5, 2, 6, 3, 7):
            wr_t = wst.tile([P, C], bf16, tag=f"wr{f}", name=f"wr{f}")
            nc.sync.dma_start(
                wr_t[:].rearrange("p (cc f) -> p cc f", cc=NCC),
                misc["wqkv_d"][:, ts(f, P)].rearrange("(cc p) f -> p cc f", p=P),
            )
            ftile[f] = wr_t

        def emit_qkv_group(f, s):
            sp = spp.tile([P, 2 * SLAB], f32, tag="sp", name="qg")
            for cc in range(NCC):
                nc.tensor.matmul(
                    sp[:, 0:SLAB],
                    ftile[f][:, ts(cc, P)],
                    xT[cc][:, ts(s, SLAB)],
                    start=(cc == 0),
                    stop=(cc == NCC - 1),
                )
            nc.vector.tensor_tensor(
                out=qkT[f][:, ts(s, SLAB)],
                in0=sp[:, 0:SLAB],
                in1=bqk[f][:].to_broadcast((P, SLAB)),
                op=Alu.add,
            )

        def emit_v_tile(tt):
            sp = spp.tile([P, 2 * SLAB], f32, tag="sp", name="vg")
            for cc in range(NCC):
                nc.tensor.matmul(
                    sp[:, 0:CL],
                    xT[cc][:, ts(tt, P)],
                    wvr_[:, ts(cc, CL)],
                    start=(cc == 0),
                    stop=(cc == NCC - 1),
                )
            vout = vaug[tt][:].rearrange("p (h e) -> p h e", e=E)
            nc.vector.tensor_tensor(
                out=vout[:, :, 0:HD],
                in0=sp[:, 0:CL].rearrange("p (h d) -> p h d", d=HD),
                in1=bvf[:].rearrange("p (h d) -> p h d", d=HD),
                op=Alu.add,
            )
            nc.vector.tensor_copy(
                vout[:, :, HD : HD + 1],
                onesf[:].unsqueeze(1).to_broadcast((P, HL, 1)),
            )

        def emit_proj(s, tts=(0, 1, 2, 3)):
            for tt2 in tts:
                for nh in range(2):
                    pps = ypp.tile([P, SLAB], f32, tag="yps", name="pps")
                    for pr2 in range(NPAIR):
                        nc.tensor.matmul(
                            pps[:],
                            ytiles[(s, pr2)][:, ts(tt2, P)],
                            wpr_[:, pr2 * C + nh * SLAB : pr2 * C + (nh + 1) * SLAB],
                            start=(pr2 == 0),
                            stop=(pr2 == NPAIR - 1),
                        )
                    ot = osb.tile([P, SLAB], f32)
                    nc.vector.tensor_copy(ot[:], pps[:])
                    nc.sync.dma_start(
                        misc["y_d"][s * SLAB + tt2 * P : s * SLAB + (tt2 + 1) * P,
                                    ts(nh, SLAB)],
                        ot[:],
                    )

        def emit_divide(ysb, yt):
            # softmax divide: row 64 of each half of ysb holds the sums.
            # reciprocal_approx_fast needs its input at base partition 0
            # (custom-DVE ops don't partition-shift), so stage it first.
            srow = dvp.tile([1, 2 * SLAB], f32, tag="srow", name="srow")
            nc.vector.tensor_copy(srow[:], ysb[64:65, :])
            sums = dvp.tile([1, 2 * SLAB], f32, tag="sums", name="sums")
            nc.vector.reciprocal_approx_fast(sums[:], srow[:])
            rb = rbp.tile([64, 2 * SLAB], f32, tag="rb", name="rb")
            nc.gpsimd.partition_broadcast(rb[:], sums[0:1, :])
            nc.vector.tensor_mul(yt[0:64, :], ysb[0:64, 0:SLAB], rb[0:64, 0:SLAB])
            for qd in range(2):
                nc.vector.tensor_mul(
                    yt[64 + 32 * qd : 96 + 32 * qd, :],
                    ysb[32 * qd : 32 * (qd + 1), SLAB : 2 * SLAB],
                    rb[32 * qd : 32 * (qd + 1), SLAB : 2 * SLAB],
                )

        ytiles = {}
        pending_div = None
        for s in range(NS):
            for tt in range(4 * s, 4 * s + 4):
                emit_v_tile(tt)
            for pr in range(NPAIR):
                if pr == 0:
                    emit_qkv_group(4, s)
                else:
                    emit_qkv_group(pr, s)
                    emit_qkv_group(4 + pr, s)
                if pending_div is not None:
                    emit_divide(*pending_div)
                    pending_div = None
                kt, qt = qkT[4 + pr], qkT[pr]
                yps = ypp.tile([P, 2 * SLAB], f32, tag="yps", name="yps")
                ntk = 4 * s + 4
                for i in range(ntk):
                    o = 0 if i < 4 * s else P * (i - 4 * s)
                    # bf16 K=64 pair: concurrent via row-group tile_position,
                    # narrowed to the causally live columns.
                    sp = spp.tile([P, 2 * SLAB], f32, tag="sp", name="sp")
                    for h2, rlo in enumerate((0, 64)):
                        nc.tensor.matmul(
                            sp[:, h2 * SLAB + o : (h2 + 1) * SLAB],
                            kt[rlo : rlo + 64, ts(i, P)],
                            qt[rlo : rlo + 64, s * SLAB + o : (s + 1) * SLAB],
                            start=True,
                            stop=True,
                            tile_position=(rlo, 0),
                        )
                    pt = ptp.tile([P, 2 * SLAB], bf16)
                    nc.scalar.activation(
                        pt[:].rearrange("p (h n) -> p h n", h=2)[:, :, o:SLAB],
                        sp[:].rearrange("p (h n) -> p h n", h=2)[:, :, o:SLAB],
                        AF.Exp,
                        scale=float(1.0 / np.sqrt(HD)),
                    )
                    if i >= 4 * s:
                        nc.vector.tensor_tensor(
                            out=pt[:].rearrange("p (h n) -> p h n", h=2)[:, :, o : o + P],
                            in0=pt[:].rearrange("p (h n) -> p h n", h=2)[:, :, o : o + P],
                            in1=mask[:].unsqueeze(1).to_broadcast((P, 2, P)),
                            op=Alu.mult,
                        )
                    for h2 in range(2):
                        nc.tensor.matmul(
                            yps[0:E, h2 * SLAB + o : (h2 + 1) * SLAB],
                            vaug[i][:, (2 * pr + h2) * E : (2 * pr + h2 + 1) * E],
                            pt[:, h2 * SLAB + o : (h2 + 1) * SLAB],
                            start=(i == 0),
                            stop=(i == ntk - 1),
                        )
                # drain the PSUM accumulator right away (frees the bank for
                # the pair after next), but defer the divide itself
                ysb = dvp.tile([E, 2 * SLAB], f32, tag="ysb", name="ysb")
                nc.vector.tensor_copy(ysb[:], yps[0:E, :])
                yt = ytp.tile([P, SLAB], bf16)
                ytiles[(s, pr)] = yt
                # the previous slab's projection is emitted BEFORE the
                # divide so its PE work never waits on the divide chain
                if s > 0:
                    emit_proj(s - 1, (pr,))
                pending_div = (ysb, yt)
        if pending_div is not None:
            emit_divide(*pending_div)
        emit_proj(NS - 1)


def _build_nc():
    _ensure_paths()
    import concourse.mybir as mybir
    import concourse.tile as tile
    from concourse import bacc
    from concourse.bass import ts
    from concourse.masks import make_identity, make_upper_triangular

    dt = mybir.dt
    f32, bf16 = dt.float32, dt.bfloat16

    nc = bacc.Bacc("TRN2", target_bir_lowering=False, debug=False)
    x_d = nc.dram_tensor("x", [T, C], bf16, kind="ExternalInput")
    wqkv_d = nc.dram_tensor("wqkv", [C, 3 * CL], bf16, kind="ExternalInput")
    bqkv_d = nc.dram_tensor("bqkv", [3 * CL], f32, kind="ExternalInput")
    wproj_d = nc.dram_tensor("wproj", [CL, C], bf16, kind="ExternalInput")
    y_d = nc.dram_tensor("y", [T, C], f32, kind="ExternalOutput")

    misc = {
        "ts": ts,
        "Alu": mybir.AluOpType,
        "AF": mybir.ActivationFunctionType,
        "x_d": x_d,
        "wqkv_d": wqkv_d,
        "wproj_d": wproj_d,
        "y_d": y_d,
    }

    with tile.TileContext(nc) as tc:
        with (
            tc.tile_pool(name="const", bufs=1) as constp,
            tc.tile_pool(name="qk", bufs=8) as qkp,
            tc.tile_pool(name="va", bufs=NTT) as vap,
        ):
            ident = constp.tile([P, P], bf16)
            make_identity(nc, ident[:])
            mask = constp.tile([P, P], bf16)
            make_upper_triangular(nc, mask[:], val=1.0, diag=True)
            onesf = constp.tile([P, 1], bf16)
            nc.vector.memset(onesf[:], 1.0)
            bvf = constp.tile([P, CL], f32)
            nc.gpsimd.dma_start(
                bvf[:], bqkv_d[2 * CL : 3 * CL].unsqueeze(0).to_broadcast((P, CL))
            )
            bqk = []
            for f in range(8):
                t_ = constp.tile([P, 1], f32, tag=f"bqk{f}", name=f"bqk{f}")
                nc.gpsimd.dma_start(t_[:], bqkv_d[f * P : (f + 1) * P].unsqueeze(1))
                bqk.append(t_)
            consts = {"ident": ident, "mask": mask, "onesf": onesf, "bvf": bvf,
                      "bqk": bqk}

            qkT = [qkp.tile([P, T], bf16, tag="qkT", name="qkT") for _ in range(8)]
            vaug = [vap.tile([P, HL * E], bf16, tag="vaug", name="vaug")
                    for _ in range(NTT)]

            with (
                tc.tile_pool(name="wpp", bufs=1) as wpp,
                tc.tile_pool(name="xtp", bufs=1) as xtp,
                tc.tile_pool(name="wvrp", bufs=1) as wvrp,
            ):
                wpt = wpp.tile([P, 4 * C], bf16)
                consts["wproj"] = wpt
                xTb = xtp.tile([P, NCC * T], bf16, tag="xT", name="xT")
                xT = [xTb[:, cc * T : (cc + 1) * T] for cc in range(NCC)]
                wvr_ = wvrp.tile([P, NCC * CL], bf16, tag="wvr", name="wvr")
                _phase1a(nc, tc, consts, qkT, xT, xTb, wvr_, (f32, bf16), misc)
                _pipeline(nc, tc, consts, qkT, vaug, xT, wvr_, (f32, bf16), misc)

    nc.compile()
    return nc


def get_nc():
    if "nc" not in _CACHE:
        _CACHE["nc"] = _build_nc()
    return _CACHE["nc"]


def _shard_inputs(x, w_attn, b_attn, w_proj):
    import ml_dtypes

    bf16 = ml_dtypes.bfloat16
    x = np.asarray(x, dtype=np.float32)
    w_attn = np.asarray(w_attn, dtype=np.float32)
    b_attn = np.asarray(b_attn, dtype=np.float32)
    w_proj = np.asarray(w_proj, dtype=np.float32)
    in_maps = []
    for c in range(B * G):
        b, g = divmod(c, G)
        sl = slice(CL * g, CL * (g + 1))
        wqkv = np.ascontiguousarray(
            np.concatenate(
                [w_attn[:, 0:C][:, sl], w_attn[:, C : 2 * C][:, sl],
                 w_attn[:, 2 * C : 3 * C][:, sl]],
                axis=1,
            ).astype(bf16)
        )
        bqkv = np.ascontiguousarray(
            np.concatenate([b_attn[0:C][sl], b_attn[C : 2 * C][sl],
                            b_attn[2 * C : 3 * C][sl]])
        )
        in_maps.append(
            {
                "x": np.ascontiguousarray(x[b].astype(bf16)),
                "wqkv": wqkv,
                "bqkv": bqkv,
                "wproj": np.ascontiguousarray(w_proj[sl, :].astype(bf16)),
            }
        )
    return in_maps


def run_spmd(x, w_attn, b_attn, w_proj, b_proj, **kwargs):
    _ensure_paths()
    from concourse.bass_utils import run_bass_kernel_spmd

    nc = get_nc()
    in_maps = _shard_inputs(x, w_attn, b_attn, w_proj)
    try:
        res = run_bass_kernel_spmd(nc, in_maps, core_ids=list(range(B * G)), **kwargs)
    except Exception:
        # transient device errors (e.g. NRT_EXEC_UNIT_UNRECOVERABLE) have
        # been observed once on this fabric; one retry recovers them
        import time as _time

        _time.sleep(2.0)
        res = run_bass_kernel_spmd(nc, in_maps, core_ids=list(range(B * G)), **kwargs)
    b_proj = np.asarray(b_proj, dtype=np.float32)
    y = np.empty((B, T, C), np.float32)
    for b in range(B):
        y[b] = res.results[G * b]["y"] + res.results[G * b + 1]["y"] + b_proj[None, :]
    return y, res


def kernel(x, w_attn, b_attn, w_proj, b_proj):
    y, _ = run_spmd(x, w_attn, b_attn, w_proj, b_proj)
    return y
